# revision 24
# baseline (speedup 1.0000x reference)
"""Trainium2 Bass kernel for masked max-pool dual-attention cosine similarity.

Problem shapes (hardcoded): v1 [128, 256, 768] f32, v2 [128, 256, 768] f32,
mask1/mask2 [128, 256] f32. Output: [128] f32 cosine similarities.

Math (equivalent to the reference):
  match = v1 @ v2^T                      [B, L1, L2]
  s1 = -max over valid l2 of match / 100 (valid l1 only)
  w1 = exp(s1) masked to 0 at invalid l1 (softmax un-normalized; cosine is
                                          scale-invariant so the 1/Z cancels)
  p1 = v1^T w1 ; p2 analogous via matchT
  out = cos(p1, p2)  (finale done on host in f64: ~128*768*3 flops)

Masking is done with additive biases so v1/v2 are used unmodified:
  - a K=1 accumulation matmul adds (mask2-1)*300 along the l2 (free) axis of
    match before the row max (excludes invalid l2 from s1's max)
  - the PSUM->SBUF copy of match adds (mask1-1)*300 per-partition before the
    PE transpose (excludes invalid l1 from s2's max)
  - the exp gets bias (mask-1)*1e4 per partition -> exp(-1e4) == 0 exactly,
    so invalid positions get weight 0

Sharding: data-parallel over batch; core c handles batches [16c, 16c+16).

Pooling matvecs (M=1) for (batch-parity, side) pairs are packed into the four
32-column PE groups (tile_position via out base partition 32j), so four of
them run concurrently and the [4, 768] result block is copied/DMA'd at once.
"""

from contextlib import ExitStack

import ml_dtypes
import numpy as np

B, L, D = 128, 256, 768
NCORES = 8
BPC = B // NCORES  # batches per core
KC = D // 128  # 6 d-chunks
LC = L // 128  # 2 l-chunks
NEG_BIAS = 300.0  # pushes masked match entries below any valid value
EXP_BIAS = 1.0e4  # exp(-1e4) == 0 in f32
SCALE = -0.01  # -1/100 from the reference

_PROGRAM_CACHE: dict = {}


def build_program(repeat=1, mode="full"):
    import concourse.tile as tile
    from concourse import bacc, mybir
    from concourse.masks import make_identity

    do_dma = mode in ("full", "dmaonly", "match", "notail")
    do_compute = mode in ("full", "nodma", "match", "notail")
    do_tail = mode in ("full", "nodma", "notail")  # transposes + s2
    do_pool = mode in ("full", "nodma")  # exps + pooling + out

    f32 = mybir.dt.float32
    bf16 = mybir.dt.bfloat16
    X = mybir.AxisListType.X
    Exp = mybir.ActivationFunctionType.Exp
    Ident = mybir.ActivationFunctionType.Identity
    sub = mybir.AluOpType.subtract
    mult = mybir.AluOpType.mult
    add = mybir.AluOpType.add

    nc = bacc.Bacc("TRN2", target_bir_lowering=False, debug=False, enable_asserts=True)

    v1t = nc.dram_tensor("v1t", [BPC, D, L], bf16, kind="ExternalInput").ap()
    v2t = nc.dram_tensor("v2t", [BPC, D, L], bf16, kind="ExternalInput").ap()
    v1n = nc.dram_tensor("v1n", [BPC, L, D], bf16, kind="ExternalInput").ap()
    v2n = nc.dram_tensor("v2n", [BPC, L, D], bf16, kind="ExternalInput").ap()
    m1d = nc.dram_tensor("m1", [BPC, L], f32, kind="ExternalInput").ap()
    m2d = nc.dram_tensor("m2", [BPC, L], f32, kind="ExternalInput").ap()
    pout = nc.dram_tensor("pout", [BPC, 2, D], f32, kind="ExternalOutput").ap()

    with tile.TileContext(nc) as tc, ExitStack() as ctx:
        const = ctx.enter_context(tc.tile_pool(name="const", bufs=1))

        ident_bf = const.tile([128, 128], bf16)
        make_identity(nc, ident_bf)
        ident_f = const.tile([128, 128], f32)
        make_identity(nc, ident_f)
        ones_col = const.tile([1, 128], bf16)
        nc.vector.memset(ones_col, 1.0)

        # masks in natural layout [BPC, L]
        msk1 = const.tile([BPC, L], f32)
        nc.sync.dma_start(msk1[:], m1d[:, :])
        msk2 = const.tile([BPC, L], f32)
        nc.sync.dma_start(msk2[:], m2d[:, :])

        # transposed masks [128(l), LC, BPC] via PE transpose
        m1T = const.tile([128, LC, BPC], f32)
        m2T = const.tile([128, LC, BPC], f32)
        with tc.tile_pool(name="ps_setup", bufs=2, space="PSUM") as ps_setup:
            for msk, mT in ((msk1, m1T), (msk2, m2T)):
                for c in range(LC):
                    tps = ps_setup.tile([128, BPC], f32, tag="tps")
                    nc.tensor.transpose(
                        tps[:], msk[:, c * 128 : (c + 1) * 128], ident_f[0:BPC, 0:BPC]
                    )
                    nc.vector.tensor_copy(mT[:, c, :], tps[:])

        # per-partition bias tiles from transposed masks
        b300_1T = const.tile([128, LC, BPC], f32)  # (m1-1)*300: into match copy
        nc.vector.tensor_scalar(
            out=b300_1T[:], in0=m1T[:], scalar1=1.0, scalar2=NEG_BIAS, op0=sub, op1=mult
        )
        b1e4T = const.tile([128, LC, BPC], f32)  # (m1-1)*1e4: exp bias side 1
        nc.vector.tensor_scalar(
            out=b1e4T[:], in0=m1T[:], scalar1=1.0, scalar2=EXP_BIAS, op0=sub, op1=mult
        )
        b2e4T = const.tile([128, LC, BPC], f32)  # (m2-1)*1e4: exp bias side 2
        nc.vector.tensor_scalar(
            out=b2e4T[:], in0=m2T[:], scalar1=1.0, scalar2=EXP_BIAS, op0=sub, op1=mult
        )

        # bias row (mask2-1)*300 as bf16 on partition 0, for the K=1 bias matmul
        m2row = const.tile([1, BPC, L], f32)
        nc.sync.dma_start(m2row[:], m2d[:, :])  # partition-gather to one row
        br2 = const.tile([1, BPC, L], bf16)
        nc.vector.tensor_scalar(
            out=br2[:], in0=m2row[:], scalar1=1.0, scalar2=NEG_BIAS, op0=sub, op1=mult
        )

        inpool = ctx.enter_context(tc.tile_pool(name="inp", bufs=5))
        msbp = ctx.enter_context(tc.tile_pool(name="msb", bufs=3))
        small = ctx.enter_context(tc.tile_pool(name="small", bufs=6))
        pvec = ctx.enter_context(tc.tile_pool(name="pvec", bufs=3))
        ps_m = ctx.enter_context(tc.tile_pool(name="ps_m", bufs=3, space="PSUM"))
        ps_t = ctx.enter_context(tc.tile_pool(name="ps_t", bufs=2, space="PSUM"))
        ps_pa = ctx.enter_context(tc.tile_pool(name="ps_pa", bufs=2, space="PSUM"))
        ps_pb = ctx.enter_context(tc.tile_pool(name="ps_pb", bufs=1, space="PSUM"))

        def batch_body():
            # 3-deep software-pipelined emission: per slot, emit the match
            # phase of batch b, transposes+max of b-1, and exp+pooling of
            # b-2, so every engine has independent work from adjacent
            # batches queued and the per-batch latency chains overlap.
            pair = {}  # parity -> (w1, w2, N1, N2)
            st = {}  # b -> live tiles between phases
            for b in range(BPC + 2):
                if b < BPC:
                    phase_match(b, st)
                if 1 <= b < BPC + 1 and do_tail:
                    phase_tail1(b - 1, st)
                if b >= 2 and do_pool:
                    phase_tail2(b - 2, st, pair)

        fixed_in = {}
        if not do_dma:
            # compute-only mode: load batch 0 once, reuse for every batch
            for nm, src, shape in (
                ("T1", v1t, [128, KC, L]),
                ("T2", v2t, [128, KC, L]),
                ("N1", v1n, [128, LC, D]),
                ("N2", v2n, [128, LC, D]),
            ):
                t = const.tile(shape, bf16, tag=f"fix_{nm}")
                pat = "(kc p) l -> p kc l" if nm[0] == "T" else "(lc p) d -> p lc d"
                nc.sync.dma_start(t[:], src[0].rearrange(pat, p=128))
                fixed_in[nm] = t

        def phase_match(b, st):
            if do_dma:
                T1 = inpool.tile([128, KC, L], bf16, tag="T1")
                nc.sync.dma_start(T1[:], v1t[b].rearrange("(kc p) l -> p kc l", p=128))
                T2 = inpool.tile([128, KC, L], bf16, tag="T2")
                nc.sync.dma_start(T2[:], v2t[b].rearrange("(kc p) l -> p kc l", p=128))
                N1 = inpool.tile([128, LC, D], bf16, tag="N1")
                nc.sync.dma_start(N1[:], v1n[b].rearrange("(lc p) d -> p lc d", p=128))
                N2 = inpool.tile([128, LC, D], bf16, tag="N2")
                nc.sync.dma_start(N2[:], v2n[b].rearrange("(lc p) d -> p lc d", p=128))
            else:
                T1, T2 = fixed_in["T1"], fixed_in["T2"]
                N1, N2 = fixed_in["N1"], fixed_in["N2"]
            if not do_compute:
                return

            # match chunks [l1-chunk mc: 128, l2: 256] = v1^T v2 + (m2-1)*300
            mps = ps_m.tile([128, LC, L], f32, tag="mps")
            for mc in range(LC):
                for k in range(KC):
                    nc.tensor.matmul(
                        mps[:, mc, :],
                        T1[:, k, mc * 128 : (mc + 1) * 128],
                        T2[:, k, :],
                        start=(k == 0),
                        stop=False,
                    )
                nc.tensor.matmul(
                    mps[:, mc, :], ones_col[:], br2[:, b, :], start=False, stop=True
                )

            s1pre = small.tile([128, LC], f32, tag="s1pre")
            nc.vector.reduce_max(out=s1pre[:], in_=mps[:], axis=X)

            # copy to SBUF (bf16) adding the per-partition l1 bias; one
            # chunk on ACT, one on DVE so the two copies run in parallel
            msb = msbp.tile([128, LC, L], bf16, tag="msb")
            nc.scalar.activation(
                out=msb[:, 0, :],
                in_=mps[:, 0, :],
                func=Ident,
                bias=b300_1T[:, 0, b : b + 1],
                scale=1.0,
            )
            nc.vector.tensor_scalar(
                out=msb[:, 1, :],
                in0=mps[:, 1, :],
                scalar1=b300_1T[:, 1, b : b + 1],
                scalar2=None,
                op0=add,
            )
            st[b] = [N1, N2, s1pre, msb]

        def phase_tail1(b, st):
            N1, N2, s1pre, msb = st[b]
            # matchT chunks via PE transpose; row max over l1
            mtps = ps_t.tile([128, LC, L], bf16, tag="mtps")
            for mc2 in range(LC):
                for mc in range(LC):
                    nc.tensor.transpose(
                        mtps[:, mc2, mc * 128 : (mc + 1) * 128],
                        msb[:, mc, mc2 * 128 : (mc2 + 1) * 128],
                        ident_bf[:],
                    )
            s2pre = small.tile([128, LC], f32, tag="s2pre")
            nc.vector.reduce_max(out=s2pre[:], in_=mtps[:], axis=X)
            st[b].append(s2pre)

        def phase_tail2(b, st, pair):
            N1, N2, s1pre, msb, s2pre = st.pop(b)
            # unnormalized attention weights w = exp(-max/100), 0 where masked
            t1v = small.tile([128, LC], f32, tag="t1v")
            nc.vector.tensor_mul(t1v[:], s1pre[:], m1T[:, :, b])
            t2v = small.tile([128, LC], f32, tag="t2v")
            nc.vector.tensor_mul(t2v[:], s2pre[:], m2T[:, :, b])
            w1 = small.tile([128, LC], bf16, tag=f"w1_{b % 2}")
            w2 = small.tile([128, LC], bf16, tag=f"w2_{b % 2}")
            for mc in range(LC):
                nc.scalar.activation(
                    out=w1[:, mc : mc + 1],
                    in_=t1v[:, mc : mc + 1],
                    func=Exp,
                    bias=b1e4T[:, mc, b : b + 1],
                    scale=SCALE,
                )
                nc.scalar.activation(
                    out=w2[:, mc : mc + 1],
                    in_=t2v[:, mc : mc + 1],
                    func=Exp,
                    bias=b2e4T[:, mc, b : b + 1],
                    scale=SCALE,
                )

            pair[b % 2] = (w1, w2, N1, N2)
            if b % 2 == 0:
                return

            # pooling for the pair (b-1, b): p = v^T w as rows of a [4, 768]
            # block on partitions {0,32,64,96} (four concurrent PE col-groups)
            pa = ps_pa.tile([128, 512], f32, tag="pa")
            pb = ps_pb.tile([128, 256], f32, tag="pb")
            for par in (0, 1):
                pw1, pw2, pn1, pn2 = pair[par]
                for s, (w, NN) in enumerate(((pw1, pn1), (pw2, pn2))):
                    j = 32 * (2 * par + s)
                    for lc in range(LC):
                        nc.tensor.matmul(
                            pa[j : j + 1, :],
                            w[:, lc : lc + 1],
                            NN[:, lc, 0:512],
                            start=(lc == 0),
                            stop=(lc == LC - 1),
                            tile_position=(0, j),
                        )
                        nc.tensor.matmul(
                            pb[j : j + 1, :],
                            w[:, lc : lc + 1],
                            NN[:, lc, 512:768],
                            start=(lc == 0),
                            stop=(lc == LC - 1),
                            tile_position=(0, j),
                        )
            # full-partition copies (engine cost scales with free dim only);
            # only rows {0,32,64,96} are meaningful, the DMA below gathers them
            p_sb = pvec.tile([128, D], f32, tag="p_sb")
            nc.scalar.copy(out=p_sb[:, 0:512], in_=pa[:])
            nc.scalar.copy(out=p_sb[:, 512:768], in_=pb[:])
            # issue on the ACT DGE: p_sb was just produced by ACT, so this
            # never stalls ACT, and it keeps the SP HWDGE FIFO loads-only
            # (a pout DMA on SP would head-of-line-block later batch loads)
            nc.scalar.dma_start(
                pout[b - 1 : b + 1].rearrange("b s d -> (b s) d"), p_sb[::32, :]
            )

        if repeat == 1:
            batch_body()
        else:
            with tc.For_i(0, repeat, 1, hint_engines=(mybir.EngineType.PE,)):
                batch_body()

    nc.compile()
    return nc


def _get_program():
    if "nc" not in _PROGRAM_CACHE:
        _PROGRAM_CACHE["nc"] = build_program()
    return _PROGRAM_CACHE["nc"]


def prepare_in_maps(v1, mask1, v2, mask2):
    bf16 = ml_dtypes.bfloat16
    v1b = v1.astype(bf16)
    v2b = v2.astype(bf16)
    v1tb = np.ascontiguousarray(v1b.transpose(0, 2, 1))
    v2tb = np.ascontiguousarray(v2b.transpose(0, 2, 1))
    m1 = np.ascontiguousarray(mask1, dtype=np.float32)
    m2 = np.ascontiguousarray(mask2, dtype=np.float32)

    in_maps = []
    for c in range(NCORES):
        sl = slice(c * BPC, (c + 1) * BPC)
        in_maps.append(
            {
                "v1t": v1tb[sl],
                "v2t": v2tb[sl],
                "v1n": v1b[sl],
                "v2n": v2b[sl],
                "m1": m1[sl],
                "m2": m2[sl],
            }
        )
    return in_maps


def finalize(results):
    """results: list of per-core dicts with 'pout' [BPC, 2, D] f32."""
    cos = np.empty(B, dtype=np.float32)
    for c in range(NCORES):
        p = results[c]["pout"].astype(np.float64)
        p1, p2 = p[:, 0, :], p[:, 1, :]
        dot = np.sum(p1 * p2, axis=-1)
        n1 = np.maximum(np.linalg.norm(p1, axis=-1), 1e-8)
        n2 = np.maximum(np.linalg.norm(p2, axis=-1), 1e-8)
        cos[c * BPC : (c + 1) * BPC] = (dot / (n1 * n2)).astype(np.float32)
    return cos


def kernel(v1, mask1, v2, mask2):
    from concourse.bass_utils import run_bass_kernel_spmd

    nc = _get_program()
    in_maps = prepare_in_maps(v1, mask1, v2, mask2)
    res = run_bass_kernel_spmd(nc, in_maps, list(range(NCORES)))
    return finalize(res.results)


if __name__ == "__main__":
    rng = np.random.default_rng(0)
    v1 = rng.standard_normal((B, L, D), dtype=np.float32)
    v2 = rng.standard_normal((B, L, D), dtype=np.float32)
    len1 = rng.integers(L // 2, L + 1, size=B)
    len2 = rng.integers(L // 2, L + 1, size=B)
    mask1 = (np.arange(L)[None, :] < len1[:, None]).astype(np.float32)
    mask2 = (np.arange(L)[None, :] < len2[:, None]).astype(np.float32)
    out = kernel(v1, mask1, v2, mask2)
    print(out[:8])


# revision 26
# speedup vs baseline: 105.8354x; 105.8354x over previous
"""Trainium2 Bass kernel for masked max-pool dual-attention cosine similarity.

Problem shapes (hardcoded): v1 [128, 256, 768] f32, v2 [128, 256, 768] f32,
mask1/mask2 [128, 256] f32. Output: [128] f32 cosine similarities.

Math (equivalent to the reference):
  match = v1 @ v2^T                      [B, L1, L2]
  s1 = -max over valid l2 of match / 100 (valid l1 only)
  w1 = exp(s1) masked to 0 at invalid l1 (softmax un-normalized; cosine is
                                          scale-invariant so the 1/Z cancels)
  p1 = v1^T w1 ; p2 analogous via matchT
  out = cos(p1, p2)  (finale done on host in f64: ~128*768*3 flops)

Masking is done with additive biases so v1/v2 are used unmodified:
  - a K=1 accumulation matmul adds (mask2-1)*300 along the l2 (free) axis of
    match before the row max (excludes invalid l2 from s1's max)
  - the PSUM->SBUF copy of match adds (mask1-1)*300 per-partition before the
    PE transpose (excludes invalid l1 from s2's max)
  - the exp gets bias (mask-1)*1e4 per partition -> exp(-1e4) == 0 exactly,
    so invalid positions get weight 0

Sharding: data-parallel over batch; core c handles batches [16c, 16c+16).

Pooling matvecs (M=1) for (batch-parity, side) pairs are packed into the four
32-column PE groups (tile_position via out base partition 32j), so four of
them run concurrently and the [4, 768] result block is copied/DMA'd at once.
"""

from contextlib import ExitStack

import ml_dtypes
import numpy as np

B, L, D = 128, 256, 768
NCORES = 8
BPC = B // NCORES  # batches per core
KC = D // 128  # 6 d-chunks
LC = L // 128  # 2 l-chunks
NEG_BIAS = 300.0  # pushes masked match entries below any valid value
EXP_BIAS = 1.0e4  # exp(-1e4) == 0 in f32
SCALE = -0.01  # -1/100 from the reference

_PROGRAM_CACHE: dict = {}


def build_program(repeat=1, mode="full"):
    import concourse.tile as tile
    from concourse import bacc, mybir
    from concourse.masks import make_identity

    do_dma = mode in ("full", "dmaonly", "match", "notail")
    do_compute = mode in ("full", "nodma", "match", "notail")
    do_tail = mode in ("full", "nodma", "notail")  # transposes + s2
    do_pool = mode in ("full", "nodma")  # exps + pooling + out

    f32 = mybir.dt.float32
    bf16 = mybir.dt.bfloat16
    X = mybir.AxisListType.X
    Exp = mybir.ActivationFunctionType.Exp
    Ident = mybir.ActivationFunctionType.Identity
    sub = mybir.AluOpType.subtract
    mult = mybir.AluOpType.mult
    add = mybir.AluOpType.add

    nc = bacc.Bacc("TRN2", target_bir_lowering=False, debug=False, enable_asserts=True)

    v1t = nc.dram_tensor("v1t", [BPC, D, L], bf16, kind="ExternalInput").ap()
    v2t = nc.dram_tensor("v2t", [BPC, D, L], bf16, kind="ExternalInput").ap()
    v1n = nc.dram_tensor("v1n", [BPC, L, D], bf16, kind="ExternalInput").ap()
    v2n = nc.dram_tensor("v2n", [BPC, L, D], bf16, kind="ExternalInput").ap()
    m1d = nc.dram_tensor("m1", [BPC, L], f32, kind="ExternalInput").ap()
    m2d = nc.dram_tensor("m2", [BPC, L], f32, kind="ExternalInput").ap()
    pout = nc.dram_tensor("pout", [BPC, 2, D], f32, kind="ExternalOutput").ap()

    with tile.TileContext(nc) as tc, ExitStack() as ctx:
        const = ctx.enter_context(tc.tile_pool(name="const", bufs=1))

        ident_bf = const.tile([128, 128], bf16)
        make_identity(nc, ident_bf)
        ident_f = const.tile([128, 128], f32)
        make_identity(nc, ident_f)
        ones_col = const.tile([1, 128], bf16)
        nc.vector.memset(ones_col, 1.0)

        # masks in natural layout [BPC, L]
        msk1 = const.tile([BPC, L], f32)
        nc.sync.dma_start(msk1[:], m1d[:, :])
        msk2 = const.tile([BPC, L], f32)
        nc.sync.dma_start(msk2[:], m2d[:, :])

        # transposed masks [128(l), LC, BPC] via PE transpose
        m1T = const.tile([128, LC, BPC], f32)
        m2T = const.tile([128, LC, BPC], f32)
        with tc.tile_pool(name="ps_setup", bufs=2, space="PSUM") as ps_setup:
            for msk, mT in ((msk1, m1T), (msk2, m2T)):
                for c in range(LC):
                    tps = ps_setup.tile([128, BPC], f32, tag="tps")
                    nc.tensor.transpose(
                        tps[:], msk[:, c * 128 : (c + 1) * 128], ident_f[0:BPC, 0:BPC]
                    )
                    nc.vector.tensor_copy(mT[:, c, :], tps[:])

        # per-partition bias tiles from transposed masks
        b300_1T = const.tile([128, LC, BPC], f32)  # (m1-1)*300: into match copy
        nc.vector.tensor_scalar(
            out=b300_1T[:], in0=m1T[:], scalar1=1.0, scalar2=NEG_BIAS, op0=sub, op1=mult
        )
        b1e4T = const.tile([128, LC, BPC], f32)  # (m1-1)*1e4: exp bias side 1
        nc.vector.tensor_scalar(
            out=b1e4T[:], in0=m1T[:], scalar1=1.0, scalar2=EXP_BIAS, op0=sub, op1=mult
        )
        b2e4T = const.tile([128, LC, BPC], f32)  # (m2-1)*1e4: exp bias side 2
        nc.vector.tensor_scalar(
            out=b2e4T[:], in0=m2T[:], scalar1=1.0, scalar2=EXP_BIAS, op0=sub, op1=mult
        )

        # bias row (mask2-1)*300 as bf16 on partition 0, for the K=1 bias matmul
        m2row = const.tile([1, BPC, L], f32)
        nc.sync.dma_start(m2row[:], m2d[:, :])  # partition-gather to one row
        br2 = const.tile([1, BPC, L], bf16)
        nc.vector.tensor_scalar(
            out=br2[:], in0=m2row[:], scalar1=1.0, scalar2=NEG_BIAS, op0=sub, op1=mult
        )

        inpool = ctx.enter_context(tc.tile_pool(name="inp", bufs=5))
        msbp = ctx.enter_context(tc.tile_pool(name="msb", bufs=3))
        small = ctx.enter_context(tc.tile_pool(name="small", bufs=6))
        pvec = ctx.enter_context(tc.tile_pool(name="pvec", bufs=3))
        ps_m = ctx.enter_context(tc.tile_pool(name="ps_m", bufs=3, space="PSUM"))
        ps_t = ctx.enter_context(tc.tile_pool(name="ps_t", bufs=2, space="PSUM"))
        ps_pa = ctx.enter_context(tc.tile_pool(name="ps_pa", bufs=2, space="PSUM"))
        ps_pb = ctx.enter_context(tc.tile_pool(name="ps_pb", bufs=1, space="PSUM"))

        noop_tile = None
        if mode == "noop":
            noop_tile = const.tile([128, 8], f32, tag="noop")

        def batch_body():
            if mode == "noop":
                nc.vector.memset(noop_tile[:], 0.0)
                return
            # 3-deep software-pipelined emission: per slot, emit the match
            # phase of batch b, transposes+max of b-1, and exp+pooling of
            # b-2, so every engine has independent work from adjacent
            # batches queued and the per-batch latency chains overlap.
            pair = {}  # parity -> (w1, w2, N1, N2)
            st = {}  # b -> live tiles between phases
            for b in range(BPC + 2):
                if b < BPC:
                    phase_match(b, st)
                if 1 <= b < BPC + 1 and do_tail:
                    phase_tail1(b - 1, st)
                if b >= 2 and do_pool:
                    phase_tail2(b - 2, st, pair)

        fixed_in = {}
        if not do_dma:
            # compute-only mode: load batch 0 once, reuse for every batch
            for nm, src, shape in (
                ("T1", v1t, [128, KC, L]),
                ("T2", v2t, [128, KC, L]),
                ("N1", v1n, [128, LC, D]),
                ("N2", v2n, [128, LC, D]),
            ):
                t = const.tile(shape, bf16, tag=f"fix_{nm}")
                pat = "(kc p) l -> p kc l" if nm[0] == "T" else "(lc p) d -> p lc d"
                nc.sync.dma_start(t[:], src[0].rearrange(pat, p=128))
                fixed_in[nm] = t

        def phase_match(b, st):
            if do_dma:
                T1 = inpool.tile([128, KC, L], bf16, tag="T1")
                nc.sync.dma_start(T1[:], v1t[b].rearrange("(kc p) l -> p kc l", p=128))
                T2 = inpool.tile([128, KC, L], bf16, tag="T2")
                nc.sync.dma_start(T2[:], v2t[b].rearrange("(kc p) l -> p kc l", p=128))
                N1 = inpool.tile([128, LC, D], bf16, tag="N1")
                nc.sync.dma_start(N1[:], v1n[b].rearrange("(lc p) d -> p lc d", p=128))
                N2 = inpool.tile([128, LC, D], bf16, tag="N2")
                nc.sync.dma_start(N2[:], v2n[b].rearrange("(lc p) d -> p lc d", p=128))
            else:
                T1, T2 = fixed_in["T1"], fixed_in["T2"]
                N1, N2 = fixed_in["N1"], fixed_in["N2"]
            if not do_compute:
                return

            # match chunks [l1-chunk mc: 128, l2: 256] = v1^T v2 + (m2-1)*300
            mps = ps_m.tile([128, LC, L], f32, tag="mps")
            for mc in range(LC):
                for k in range(KC):
                    nc.tensor.matmul(
                        mps[:, mc, :],
                        T1[:, k, mc * 128 : (mc + 1) * 128],
                        T2[:, k, :],
                        start=(k == 0),
                        stop=False,
                    )
                nc.tensor.matmul(
                    mps[:, mc, :], ones_col[:], br2[:, b, :], start=False, stop=True
                )

            s1pre = small.tile([128, LC], f32, tag="s1pre")
            nc.vector.reduce_max(out=s1pre[:], in_=mps[:], axis=X)

            # copy to SBUF (bf16) adding the per-partition l1 bias; one
            # chunk on ACT, one on DVE so the two copies run in parallel
            msb = msbp.tile([128, LC, L], bf16, tag="msb")
            nc.scalar.activation(
                out=msb[:, 0, :],
                in_=mps[:, 0, :],
                func=Ident,
                bias=b300_1T[:, 0, b : b + 1],
                scale=1.0,
            )
            nc.vector.tensor_scalar(
                out=msb[:, 1, :],
                in0=mps[:, 1, :],
                scalar1=b300_1T[:, 1, b : b + 1],
                scalar2=None,
                op0=add,
            )
            st[b] = [N1, N2, s1pre, msb]

        def phase_tail1(b, st):
            N1, N2, s1pre, msb = st[b]
            # matchT chunks via PE transpose; row max over l1
            mtps = ps_t.tile([128, LC, L], bf16, tag="mtps")
            for mc2 in range(LC):
                for mc in range(LC):
                    nc.tensor.transpose(
                        mtps[:, mc2, mc * 128 : (mc + 1) * 128],
                        msb[:, mc, mc2 * 128 : (mc2 + 1) * 128],
                        ident_bf[:],
                    )
            s2pre = small.tile([128, LC], f32, tag="s2pre")
            nc.vector.reduce_max(out=s2pre[:], in_=mtps[:], axis=X)
            st[b].append(s2pre)

        def phase_tail2(b, st, pair):
            N1, N2, s1pre, msb, s2pre = st.pop(b)
            # unnormalized attention weights w = exp(-max/100), 0 where masked
            t1v = small.tile([128, LC], f32, tag="t1v")
            nc.vector.tensor_mul(t1v[:], s1pre[:], m1T[:, :, b])
            t2v = small.tile([128, LC], f32, tag="t2v")
            nc.vector.tensor_mul(t2v[:], s2pre[:], m2T[:, :, b])
            w1 = small.tile([128, LC], bf16, tag=f"w1_{b % 2}")
            w2 = small.tile([128, LC], bf16, tag=f"w2_{b % 2}")
            for mc in range(LC):
                nc.scalar.activation(
                    out=w1[:, mc : mc + 1],
                    in_=t1v[:, mc : mc + 1],
                    func=Exp,
                    bias=b1e4T[:, mc, b : b + 1],
                    scale=SCALE,
                )
                nc.scalar.activation(
                    out=w2[:, mc : mc + 1],
                    in_=t2v[:, mc : mc + 1],
                    func=Exp,
                    bias=b2e4T[:, mc, b : b + 1],
                    scale=SCALE,
                )

            pair[b % 2] = (w1, w2, N1, N2)
            if b % 2 == 0:
                return

            # pooling for the pair (b-1, b): p = v^T w as rows of a [4, 768]
            # block on partitions {0,32,64,96} (four concurrent PE col-groups)
            pa = ps_pa.tile([128, 512], f32, tag="pa")
            pb = ps_pb.tile([128, 256], f32, tag="pb")
            for par in (0, 1):
                pw1, pw2, pn1, pn2 = pair[par]
                for s, (w, NN) in enumerate(((pw1, pn1), (pw2, pn2))):
                    j = 32 * (2 * par + s)
                    for lc in range(LC):
                        nc.tensor.matmul(
                            pa[j : j + 1, :],
                            w[:, lc : lc + 1],
                            NN[:, lc, 0:512],
                            start=(lc == 0),
                            stop=(lc == LC - 1),
                            tile_position=(0, j),
                        )
                        nc.tensor.matmul(
                            pb[j : j + 1, :],
                            w[:, lc : lc + 1],
                            NN[:, lc, 512:768],
                            start=(lc == 0),
                            stop=(lc == LC - 1),
                            tile_position=(0, j),
                        )
            # full-partition copies (engine cost scales with free dim only);
            # only rows {0,32,64,96} are meaningful, the DMA below gathers them
            p_sb = pvec.tile([128, D], f32, tag="p_sb")
            nc.scalar.copy(out=p_sb[:, 0:512], in_=pa[:])
            nc.scalar.copy(out=p_sb[:, 512:768], in_=pb[:])
            # issue on the ACT DGE: p_sb was just produced by ACT, so this
            # never stalls ACT, and it keeps the SP HWDGE FIFO loads-only
            # (a pout DMA on SP would head-of-line-block later batch loads)
            nc.scalar.dma_start(
                pout[b - 1 : b + 1].rearrange("b s d -> (b s) d"), p_sb[::32, :]
            )

        if repeat == 1:
            batch_body()
        else:
            with tc.For_i(0, repeat, 1, hint_engines=(mybir.EngineType.PE,)):
                batch_body()

    nc.compile()
    return nc


def _get_program():
    if "nc" not in _PROGRAM_CACHE:
        _PROGRAM_CACHE["nc"] = build_program()
    return _PROGRAM_CACHE["nc"]


def prepare_in_maps(v1, mask1, v2, mask2):
    bf16 = ml_dtypes.bfloat16
    v1b = v1.astype(bf16)
    v2b = v2.astype(bf16)
    v1tb = np.ascontiguousarray(v1b.transpose(0, 2, 1))
    v2tb = np.ascontiguousarray(v2b.transpose(0, 2, 1))
    m1 = np.ascontiguousarray(mask1, dtype=np.float32)
    m2 = np.ascontiguousarray(mask2, dtype=np.float32)

    in_maps = []
    for c in range(NCORES):
        sl = slice(c * BPC, (c + 1) * BPC)
        in_maps.append(
            {
                "v1t": v1tb[sl],
                "v2t": v2tb[sl],
                "v1n": v1b[sl],
                "v2n": v2b[sl],
                "m1": m1[sl],
                "m2": m2[sl],
            }
        )
    return in_maps


def finalize(results):
    """results: list of per-core dicts with 'pout' [BPC, 2, D] f32."""
    cos = np.empty(B, dtype=np.float32)
    for c in range(NCORES):
        p = results[c]["pout"].astype(np.float64)
        p1, p2 = p[:, 0, :], p[:, 1, :]
        dot = np.sum(p1 * p2, axis=-1)
        n1 = np.maximum(np.linalg.norm(p1, axis=-1), 1e-8)
        n2 = np.maximum(np.linalg.norm(p2, axis=-1), 1e-8)
        cos[c * BPC : (c + 1) * BPC] = (dot / (n1 * n2)).astype(np.float32)
    return cos


def kernel(v1, mask1, v2, mask2):
    from concourse.bass_utils import run_bass_kernel_spmd

    nc = _get_program()
    in_maps = prepare_in_maps(v1, mask1, v2, mask2)
    res = run_bass_kernel_spmd(nc, in_maps, list(range(NCORES)))
    return finalize(res.results)


if __name__ == "__main__":
    rng = np.random.default_rng(0)
    v1 = rng.standard_normal((B, L, D), dtype=np.float32)
    v2 = rng.standard_normal((B, L, D), dtype=np.float32)
    len1 = rng.integers(L // 2, L + 1, size=B)
    len2 = rng.integers(L // 2, L + 1, size=B)
    mask1 = (np.arange(L)[None, :] < len1[:, None]).astype(np.float32)
    mask2 = (np.arange(L)[None, :] < len2[:, None]).astype(np.float32)
    out = kernel(v1, mask1, v2, mask2)
    print(out[:8])
